# revision 1
# baseline (speedup 1.0000x reference)
"""KGAT 2-layer GNN message passing on 8 trn2 NeuronCores (Bass/Tile).

Sharding: destination-row partition. Each core owns 20000 destination rows and
the edges pointing into them. Edges are bucketed (host) into 128-row blocks;
per block the kernel gathers source embeddings with one indirect DMA, builds a
val-weighted one-hot per 128-edge tile (one fused DVE tensor_scalar), and
accumulates side^T = sum val * x[col]^T via PSUM matmuls. Bi-interaction MLP +
L2-normalize run per block. The inter-layer all-gather of ego1 happens on the
host between the two layer NEFFs.
"""
import numpy as np

import concourse.bass as bass
import concourse.mybir as mybir
import concourse.tile as tile
from concourse import bacc
from concourse.bass_utils import run_bass_kernel_spmd
from concourse.masks import make_identity

N = 160000
E = 2560000
NC = 8
SHARD = N // NC          # 20000
BW = 128                 # dest block width
NBLK = -(-SHARD // BW)   # 157 (last block has 32 rows)
LAST_ROWS = SHARD - (NBLK - 1) * BW  # 32

F32 = mybir.dt.float32
I32 = mybir.dt.int32

_cache = {}
LAST_EXEC_NS = None
_TRACE = bool(__import__("os").environ.get("KGAT_TRACE"))


def _prep_edges(edge_row, edge_col, edge_val):
    """Bucket edges by (core, block); pad each block to T tiles of 128 slots.

    Returns meta arrays per core shaped [128, NBLK*3T] fp32 where block b's
    slice [:, b*3T:(b+1)*3T] holds [idx(int32 bits) | rowlocal fp32 | val fp32].
    """
    core = edge_row // SHARD
    rloc = edge_row - core * SHARD
    blk = rloc // BW
    rowlocal = rloc - blk * BW
    key = core * NBLK + blk

    order = np.argsort(key, kind="stable")
    key_s = key[order]
    col_s = edge_col[order].astype(np.int32)
    rl_s = rowlocal[order].astype(np.float32)
    val_s = edge_val[order].astype(np.float32)

    counts = np.bincount(key_s, minlength=NC * NBLK)
    T = int(-(-counts.max() // 128))
    starts = np.concatenate([[0], np.cumsum(counts)[:-1]])
    rank = np.arange(E) - starts[key_s]  # position within block

    S = T * 128
    # slot layout within a block: slot = t*128 + lane ; meta wants [lane, t]
    t_idx = rank // 128
    lane = rank % 128

    idx_a = np.zeros((NC, NBLK, 128, T), np.int32)
    rl_a = np.zeros((NC, NBLK, 128, T), np.float32)
    val_a = np.zeros((NC, NBLK, 128, T), np.float32)
    c_s = key_s // NBLK
    b_s = key_s % NBLK
    idx_a[c_s, b_s, lane, t_idx] = col_s
    rl_a[c_s, b_s, lane, t_idx] = rl_s
    val_a[c_s, b_s, lane, t_idx] = val_s

    # per core: midx [128, NBLK*T] int32 ; mval [128, NBLK*2T] f32
    midx = np.ascontiguousarray(idx_a.transpose(0, 2, 1, 3).reshape(NC, 128, NBLK * T))
    mval = np.empty((NC, NBLK, 128, 2 * T), np.float32)
    mval[:, :, :, 0:T] = rl_a
    mval[:, :, :, T:] = val_a
    mval = np.ascontiguousarray(mval.transpose(0, 2, 1, 3).reshape(NC, 128, NBLK * 2 * T))
    return midx, mval, T


def _build_layer(D, DO, T, emit_ego):
    """Build one layer's Bacc program.

    D: input embed dim; DO: output dim; emit_ego: also output unnormalized ego
    rows (needed between layers).
    """
    nc = bacc.Bacc("TRN2", target_bir_lowering=False, debug=False, num_devices=NC)
    x_full = nc.dram_tensor("x_full", [N, D], F32, kind="ExternalInput")
    xT = nc.dram_tensor("xT", [D, SHARD], F32, kind="ExternalInput")
    midx = nc.dram_tensor("midx", [128, NBLK * T], I32, kind="ExternalInput")
    mval = nc.dram_tensor("mval", [128, NBLK * 2 * T], F32, kind="ExternalInput")
    w1 = nc.dram_tensor("w1", [D, DO], F32, kind="ExternalInput")
    w2 = nc.dram_tensor("w2", [D, DO], F32, kind="ExternalInput")
    b1 = nc.dram_tensor("b1", [DO, 1], F32, kind="ExternalInput")
    b2 = nc.dram_tensor("b2", [DO, 1], F32, kind="ExternalInput")
    norm_out = nc.dram_tensor("norm_out", [SHARD, DO], F32, kind="ExternalOutput")
    if emit_ego:
        ego_out = nc.dram_tensor("ego_out", [SHARD, DO], F32, kind="ExternalOutput")

    with tile.TileContext(nc) as tc:
        with tc.tile_pool(name="const", bufs=1) as cp, \
             tc.tile_pool(name="meta", bufs=6) as mp, \
             tc.tile_pool(name="gath", bufs=4) as gp, \
             tc.tile_pool(name="onehot", bufs=6) as op_, \
             tc.tile_pool(name="work", bufs=4) as wp, \
             tc.tile_pool(name="ps", bufs=2, space="PSUM") as pp, \
             tc.tile_pool(name="ps2", bufs=2, space="PSUM") as pp2:
            iota_i = cp.tile([128, 128], I32)
            nc.gpsimd.iota(iota_i[:], pattern=[[1, 128]], base=0, channel_multiplier=0)
            iota_f = cp.tile([128, 128], F32)
            nc.vector.tensor_copy(iota_f[:], iota_i[:])
            ident = cp.tile([DO, DO], F32)
            make_identity(nc, ident[:])
            w1_t = cp.tile([D, DO], F32)
            nc.sync.dma_start(w1_t[:], w1[:, :])
            w2_t = cp.tile([D, DO], F32)
            nc.sync.dma_start(w2_t[:], w2[:, :])
            b1_t = cp.tile([DO, 1], F32)
            nc.sync.dma_start(b1_t[:], b1[:, :])
            b2_t = cp.tile([DO, 1], F32)
            nc.sync.dma_start(b2_t[:], b2[:, :])

            for b in range(NBLK):
                rows = BW if b < NBLK - 1 else LAST_ROWS
                it = mp.tile([128, T], I32, tag="it")
                nc.sync.dma_start(it[:], midx[:, b * T : (b + 1) * T])
                mt = mp.tile([128, 2 * T], F32, tag="mt")
                nc.sync.dma_start(mt[:], mval[:, b * 2 * T : (b + 1) * 2 * T])

                xg = gp.tile([128, T * D], F32, tag="xg")
                for t in range(T):
                    nc.gpsimd.indirect_dma_start(
                        out=xg[:, t * D : (t + 1) * D], out_offset=None, in_=x_full[:, :],
                        in_offset=bass.IndirectOffsetOnAxis(ap=it[:, t : t + 1], axis=0),
                    )

                egoT = wp.tile([D, BW], F32, tag="egoT")
                nc.sync.dma_start(egoT[:, :rows], xT[:, b * BW : b * BW + rows])

                sideT_ps = pp.tile([D, BW], F32, space="PSUM", tag="sideT")
                for t in range(T):
                    P = op_.tile([128, 128], F32, tag="P")
                    nc.vector.tensor_scalar(
                        out=P[:], in0=iota_f[:],
                        scalar1=mt[:, t : t + 1],
                        scalar2=mt[:, T + t : T + t + 1],
                        op0=mybir.AluOpType.is_equal,
                        op1=mybir.AluOpType.mult,
                    )
                    nc.tensor.matmul(
                        out=sideT_ps[:], lhsT=xg[:, t * D : (t + 1) * D], rhs=P[:],
                        start=(t == 0), stop=(t == T - 1),
                    )

                sumT = wp.tile([D, BW], F32, tag="sumT")
                nc.vector.tensor_tensor(
                    out=sumT[:, :rows], in0=egoT[:, :rows], in1=sideT_ps[:, :rows],
                    op=mybir.AluOpType.add)
                prodT = wp.tile([D, BW], F32, tag="prodT")
                nc.vector.tensor_tensor(
                    out=prodT[:, :rows], in0=egoT[:, :rows], in1=sideT_ps[:, :rows],
                    op=mybir.AluOpType.mult)

                h1_ps = pp2.tile([DO, BW], F32, space="PSUM", tag="h1")
                nc.tensor.matmul(out=h1_ps[:, :rows], lhsT=w1_t[:], rhs=sumT[:, :rows],
                                 start=True, stop=True)
                h2_ps = pp2.tile([DO, BW], F32, space="PSUM", tag="h2")
                nc.tensor.matmul(out=h2_ps[:, :rows], lhsT=w2_t[:], rhs=prodT[:, :rows],
                                 start=True, stop=True)
                h1 = wp.tile([DO, BW], F32, tag="h1s")
                nc.scalar.activation(out=h1[:, :rows], in_=h1_ps[:, :rows],
                                     func=mybir.ActivationFunctionType.Lrelu,
                                     bias=b1_t[:], scale=1.0, alpha=0.01)
                h2 = wp.tile([DO, BW], F32, tag="h2s")
                nc.scalar.activation(out=h2[:, :rows], in_=h2_ps[:, :rows],
                                     func=mybir.ActivationFunctionType.Lrelu,
                                     bias=b2_t[:], scale=1.0, alpha=0.01)
                egoNT = wp.tile([DO, BW], F32, tag="egoNT")
                nc.vector.tensor_tensor(out=egoNT[:, :rows], in0=h1[:, :rows],
                                        in1=h2[:, :rows], op=mybir.AluOpType.add)

                ego_ps = pp2.tile([BW, DO], F32, space="PSUM", tag="egor")
                nc.tensor.transpose(out=ego_ps[:rows, :], in_=egoNT[:, :rows],
                                    identity=ident[:])
                ego_r = wp.tile([BW, DO], F32, tag="egor_s")
                nc.vector.tensor_copy(ego_r[:rows, :], ego_ps[:rows, :])
                if emit_ego:
                    nc.sync.dma_start(ego_out[b * BW : b * BW + rows, :], ego_r[:rows, :])

                sq = wp.tile([BW, DO], F32, tag="sq")
                ss = wp.tile([BW, 1], F32, tag="ss")
                nc.scalar.activation(out=sq[:rows, :], in_=ego_r[:rows, :],
                                     func=mybir.ActivationFunctionType.Square,
                                     accum_out=ss[:rows, :])
                nrm = wp.tile([BW, 1], F32, tag="nrm")
                nc.scalar.sqrt(nrm[:rows, :], ss[:rows, :])
                nc.vector.tensor_scalar_max(nrm[:rows, :], nrm[:rows, :], 1e-12)
                rinv = wp.tile([BW, 1], F32, tag="rinv")
                nc.vector.reciprocal(rinv[:rows, :], nrm[:rows, :])
                nr = wp.tile([BW, DO], F32, tag="nr")
                nc.vector.tensor_scalar_mul(nr[:rows, :], ego_r[:rows, :], rinv[:rows, :])
                nc.sync.dma_start(norm_out[b * BW : b * BW + rows, :], nr[:rows, :])

    nc.compile()
    return nc


def kernel(node_embed, edge_row, edge_col, edge_val,
           W1_0, b1_0, W2_0, b2_0, W1_1, b1_1, W2_1, b2_1):
    node_embed = np.asarray(node_embed, np.float32)
    edge_row = np.asarray(edge_row, np.int32)
    edge_col = np.asarray(edge_col, np.int32)
    edge_val = np.asarray(edge_val, np.float32)

    midx, mval, T = _prep_edges(edge_row, edge_col, edge_val)

    key0 = ("L0", T)
    if key0 not in _cache:
        _cache[key0] = _build_layer(64, 32, T, emit_ego=True)
    if ("L1", T) not in _cache:
        _cache[("L1", T)] = _build_layer(32, 16, T, emit_ego=False)
    nc0 = _cache[key0]
    nc1 = _cache[("L1", T)]

    x0 = np.ascontiguousarray(node_embed)
    in_maps0 = []
    for c in range(NC):
        in_maps0.append({
            "x_full": x0,
            "xT": np.ascontiguousarray(x0[c * SHARD : (c + 1) * SHARD].T),
            "midx": midx[c], "mval": mval[c],
            "w1": np.ascontiguousarray(W1_0, dtype=np.float32),
            "w2": np.ascontiguousarray(W2_0, dtype=np.float32),
            "b1": np.ascontiguousarray(np.asarray(b1_0, np.float32).reshape(-1, 1)),
            "b2": np.ascontiguousarray(np.asarray(b2_0, np.float32).reshape(-1, 1)),
        })
    res0 = run_bass_kernel_spmd(nc0, in_maps0, core_ids=list(range(NC)), trace=_TRACE)

    ego1 = np.concatenate([res0.results[c]["ego_out"] for c in range(NC)], axis=0)
    norm1 = np.concatenate([res0.results[c]["norm_out"] for c in range(NC)], axis=0)

    x1 = np.ascontiguousarray(ego1)
    in_maps1 = []
    for c in range(NC):
        in_maps1.append({
            "x_full": x1,
            "xT": np.ascontiguousarray(x1[c * SHARD : (c + 1) * SHARD].T),
            "midx": midx[c], "mval": mval[c],
            "w1": np.ascontiguousarray(W1_1, dtype=np.float32),
            "w2": np.ascontiguousarray(W2_1, dtype=np.float32),
            "b1": np.ascontiguousarray(np.asarray(b1_1, np.float32).reshape(-1, 1)),
            "b2": np.ascontiguousarray(np.asarray(b2_1, np.float32).reshape(-1, 1)),
        })
    res1 = run_bass_kernel_spmd(nc1, in_maps1, core_ids=list(range(NC)), trace=_TRACE)
    norm2 = np.concatenate([res1.results[c]["norm_out"] for c in range(NC)], axis=0)

    global LAST_EXEC_NS
    if res0.exec_time_ns is not None or res1.exec_time_ns is not None:
        LAST_EXEC_NS = (res0.exec_time_ns or 0) + (res1.exec_time_ns or 0)
        globals()["LAST_RES"] = (res0, res1)

    out = np.empty((N, 64 + 32 + 16), np.float32)
    out[:, :64] = node_embed
    out[:, 64:96] = norm1
    out[:, 96:] = norm2
    return out



# revision 5
# speedup vs baseline: 1.0496x; 1.0496x over previous
"""KGAT 2-layer GNN message passing on 8 trn2 NeuronCores (Bass/Tile).

Sharding: destination-row partition. Each core owns 20000 destination rows and
the edges pointing into them. Host buckets edges by (128-row dest block,
32768-row source window) into 128-edge tiles. Per group of G blocks the kernel
issues one dma_gather per source window (custom SWDGE firmware: int16 local
indices, 256B rows = bf16 embeddings duplicated to 128 elems), then per tile
builds a val-weighted one-hot (fused DVE tensor_scalar, bf16) and accumulates
side^T = sum val * x[col]^T via bf16 PSUM matmuls. Bi-interaction MLP runs per
block (bf16 matmuls, fp32 PSUM); L2-normalize is batched at the layer end to
avoid ACT table thrashing. The inter-layer exchange of ego1 (bf16) happens on
the host between the two layer NEFFs.
"""
import numpy as np
import ml_dtypes

import concourse.bass as bass
import concourse.mybir as mybir
import concourse.tile as tile
from concourse import bacc
from concourse.bass_utils import run_bass_kernel_spmd
from concourse.masks import make_identity

N = 160000
E = 2560000
NC = 8
SHARD = N // NC          # 20000
BW = 128                 # dest block width
NBLK = -(-SHARD // BW)   # 157 (last block has 32 rows)
LAST_ROWS = SHARD - (NBLK - 1) * BW  # 32
WIN = 32768              # source window (int16 index range)
NW = -(-N // WIN)        # 5
G = 6                    # dest blocks per gather group

F32 = mybir.dt.float32
BF16 = mybir.dt.bfloat16
I16 = mybir.dt.int16
BF_NP = ml_dtypes.bfloat16

_cache = {}
LAST_EXEC_NS = None
_TRACE = bool(__import__("os").environ.get("KGAT_TRACE"))


def _prep_edges(edge_row, edge_col, edge_val):
    """Bucket edges by (core, dest block, source window) into 128-edge tiles.

    Tile counts are shared across cores (max over cores) so one SPMD program
    fits all. Returns:
      idx16  [NC, 128, 8*TOT] int16 — window-local column indices packed for
             dma_gather (slot k of a call at [k%16, 8*call_base + k//16],
             replicated 8x across partitions)
      mval   [NC, 128, 2*TOT] fp32 — per global tile j: col 2j rowlocal,
             col 2j+1 edge value (0 for padding slots)
      groups — per group: dict(gt0, tiles_g, calls=[(w, base_in_g, n)],
             blocks=[(b, [(base_in_g, t_bw, j0), ...]), ...])
    """
    core = edge_row // SHARD
    rloc = edge_row - core * SHARD
    blk = rloc // BW
    rowlocal = (rloc - blk * BW).astype(np.float32)
    win = edge_col // WIN
    colloc = (edge_col - win * WIN).astype(np.int16)

    key = (core * NBLK + blk) * NW + win
    order = np.argsort(key, kind="stable")
    key_s = key[order]
    loc_s = colloc[order]
    rl_s = rowlocal[order]
    val_s = edge_val[order].astype(np.float32)
    c_s = key_s // (NBLK * NW)
    b_s = (key_s // NW) % NBLK
    w_s = key_s % NW

    counts = np.bincount(key_s, minlength=NC * NBLK * NW).reshape(NC, NBLK, NW)
    t_bw = -(-counts.max(axis=0) // 128)          # [NBLK, NW]
    # ensure every block has at least one tile (PSUM needs >=1 matmul)
    empty = t_bw.sum(axis=1) == 0
    t_bw[empty, 0] = 1

    # global tile order: group g -> window w -> block b in g -> tiles
    j0 = np.zeros((NBLK, NW), np.int64)           # global tile base per (b,w)
    cb = np.zeros((NBLK, NW), np.int64)           # tile base within group
    groups = []
    jg = 0
    for g0 in range(0, NBLK, G):
        blks = range(g0, min(g0 + G, NBLK))
        gt0 = jg
        calls = []
        base = 0
        for w in range(NW):
            n = 0
            for b in blks:
                j0[b, w] = jg
                cb[b, w] = base + n
                n += int(t_bw[b, w])
                jg += int(t_bw[b, w])
            if n > 0:
                calls.append((w, base, n))
            base += n
        blocks = []
        for b in blks:
            segs = [(int(cb[b, w]), int(t_bw[b, w]), int(j0[b, w]))
                    for w in range(NW) if t_bw[b, w] > 0]
            blocks.append((b, segs))
        groups.append(dict(gt0=gt0, tiles_g=base, calls=calls, blocks=blocks))
    TOT = jg

    # per-edge slot positions
    starts = np.concatenate([[0], np.cumsum(counts.ravel())[:-1]])
    rank = np.arange(E) - starts[key_s]
    tloc = rank // 128
    lane = rank % 128

    jj = j0[b_s, w_s] + tloc                      # global tile per edge
    mval = np.zeros((NC, 128, 2 * TOT), np.float32)
    mval[c_s, lane, 2 * jj] = rl_s
    mval[c_s, lane, 2 * jj + 1] = val_s

    k_call = (cb[b_s, w_s] + tloc) * 128 + lane   # slot within the (g,w) call
    gt0_of_j = np.zeros(TOT, np.int64)
    for grp in groups:
        gt0_of_j[grp["gt0"] : grp["gt0"] + grp["tiles_g"]] = grp["gt0"]
    # column in the packed idx array: 8*(gt0 + k//16 ... ) careful:
    # packed col = 8*gt0_of_group + k_call//16 ; row = k_call%16
    idx16 = np.zeros((NC, 16, 8 * TOT), np.int16)
    col = 8 * gt0_of_j[jj] + k_call // 16
    idx16[c_s, k_call % 16, col] = loc_s
    idx16 = np.ascontiguousarray(np.tile(idx16, (1, 8, 1)))

    mval = np.ascontiguousarray(mval)
    return idx16, mval, groups, TOT


def _build_layer(D, DO, groups, TOT, emit_ego):
    """Build one layer's Bacc program (SPMD across NC cores)."""
    nc = bacc.Bacc("TRN2", target_bir_lowering=False, debug=False, num_devices=NC)
    x2 = nc.dram_tensor("x2", [N, 128], BF16, kind="ExternalInput")
    xT = nc.dram_tensor("xT", [D, SHARD], BF16, kind="ExternalInput")
    idx16 = nc.dram_tensor("idx16", [128, 8 * TOT], I16, kind="ExternalInput")
    mval = nc.dram_tensor("mval", [128, 2 * TOT], F32, kind="ExternalInput")
    w1 = nc.dram_tensor("w1", [D, DO], BF16, kind="ExternalInput")
    w2 = nc.dram_tensor("w2", [D, DO], BF16, kind="ExternalInput")
    b1 = nc.dram_tensor("b1", [DO, 1], F32, kind="ExternalInput")
    b2 = nc.dram_tensor("b2", [DO, 1], F32, kind="ExternalInput")
    norm_out = nc.dram_tensor("norm_out", [SHARD, DO], F32, kind="ExternalOutput")
    if emit_ego:
        ego_out = nc.dram_tensor("ego_out", [SHARD, DO], BF16, kind="ExternalOutput")

    TG = max(g["tiles_g"] for g in groups)
    NFULL = NBLK - 1

    with tile.TileContext(nc) as tc:
        with tc.tile_pool(name="const", bufs=1) as cp, \
             tc.tile_pool(name="idxp", bufs=2) as ip, \
             tc.tile_pool(name="gath", bufs=2) as gp, \
             tc.tile_pool(name="onehot", bufs=4) as op_, \
             tc.tile_pool(name="ego", bufs=3) as ep, \
             tc.tile_pool(name="work", bufs=4) as wp, \
             tc.tile_pool(name="ps", bufs=2, space="PSUM") as pp, \
             tc.tile_pool(name="ps2", bufs=2, space="PSUM") as pp2:
            iota_i = cp.tile([128, 128], I16)
            nc.gpsimd.iota(iota_i[:], pattern=[[1, 128]], base=0, channel_multiplier=0)
            iota_b = cp.tile([128, 128], BF16)
            nc.vector.tensor_copy(iota_b[:], iota_i[:])
            ident = cp.tile([DO, DO], F32)
            make_identity(nc, ident[:])
            w1_t = cp.tile([D, DO], BF16)
            nc.sync.dma_start(w1_t[:], w1[:, :])
            w2_t = cp.tile([D, DO], BF16)
            nc.sync.dma_start(w2_t[:], w2[:, :])
            b1_t = cp.tile([DO, 1], F32)
            nc.sync.dma_start(b1_t[:], b1[:, :])
            b2_t = cp.tile([DO, 1], F32)
            nc.sync.dma_start(b2_t[:], b2[:, :])
            mval_t = cp.tile([128, 2 * TOT], F32)
            nc.sync.dma_start(mval_t[:], mval[:, :])
            stage_e = cp.tile([128, NBLK * DO], BF16)
            stage_n = cp.tile([128, NBLK * DO], F32)
            ss = cp.tile([128, NBLK], F32)
            nrm = cp.tile([128, NBLK], F32)
            rinv = cp.tile([128, NBLK], F32)

            for grp in groups:
                gt0 = grp["gt0"]
                tiles_g = grp["tiles_g"]
                idx_g = ip.tile([128, 8 * TG], I16, tag="idx")
                nc.sync.dma_start(idx_g[:, : 8 * tiles_g],
                                  idx16[:, 8 * gt0 : 8 * (gt0 + tiles_g)])
                xg = gp.tile([128, TG * 128], BF16, tag="xg")
                for (w, base, n) in grp["calls"]:
                    w0 = w * WIN
                    w1r = min(N, w0 + WIN)
                    nc.gpsimd.dma_gather(
                        out_ap=xg[:, base * 128 : (base + n) * 128].rearrange(
                            "p (t e) -> p t e", e=128),
                        in_ap=x2[w0:w1r, :],
                        idxs_ap=idx_g[:, 8 * base : 8 * (base + n)],
                        num_idxs=128 * n,
                        num_idxs_reg=128 * n,
                        elem_size=128,
                    )

                for (b, segs) in grp["blocks"]:
                    rows = BW if b < NBLK - 1 else LAST_ROWS
                    egoT = ep.tile([D, BW], BF16, tag="egoT")
                    nc.sync.dma_start(egoT[:, :rows], xT[:, b * BW : b * BW + rows])

                    sideT_ps = pp.tile([D, BW], F32, space="PSUM", tag="sideT")
                    nops = sum(t for (_, t, _) in segs)
                    k = 0
                    for (base, t_bw, j0) in segs:
                        for ti in range(t_bw):
                            j = j0 + ti
                            P = op_.tile([128, 128], BF16, tag="P")
                            nc.vector.tensor_scalar(
                                out=P[:], in0=iota_b[:],
                                scalar1=mval_t[:, 2 * j : 2 * j + 1],
                                scalar2=mval_t[:, 2 * j + 1 : 2 * j + 2],
                                op0=mybir.AluOpType.is_equal,
                                op1=mybir.AluOpType.mult,
                            )
                            nc.tensor.matmul(
                                out=sideT_ps[:],
                                lhsT=xg[:, (base + ti) * 128 : (base + ti) * 128 + D],
                                rhs=P[:],
                                start=(k == 0), stop=(k == nops - 1),
                            )
                            k += 1

                    sumT = wp.tile([D, BW], BF16, tag="sumT")
                    nc.vector.tensor_tensor(
                        out=sumT[:, :rows], in0=egoT[:, :rows], in1=sideT_ps[:, :rows],
                        op=mybir.AluOpType.add)
                    prodT = wp.tile([D, BW], BF16, tag="prodT")
                    nc.vector.tensor_tensor(
                        out=prodT[:, :rows], in0=egoT[:, :rows], in1=sideT_ps[:, :rows],
                        op=mybir.AluOpType.mult)

                    h1_ps = pp2.tile([DO, BW], F32, space="PSUM", tag="h1")
                    nc.tensor.matmul(out=h1_ps[:, :rows], lhsT=w1_t[:],
                                     rhs=sumT[:, :rows], start=True, stop=True)
                    h2_ps = pp2.tile([DO, BW], F32, space="PSUM", tag="h2")
                    nc.tensor.matmul(out=h2_ps[:, :rows], lhsT=w2_t[:],
                                     rhs=prodT[:, :rows], start=True, stop=True)
                    h1 = wp.tile([DO, BW], F32, tag="h1s")
                    nc.scalar.activation(out=h1[:, :rows], in_=h1_ps[:, :rows],
                                         func=mybir.ActivationFunctionType.Lrelu,
                                         bias=b1_t[:], scale=1.0, alpha=0.01)
                    h2 = wp.tile([DO, BW], F32, tag="h2s")
                    nc.scalar.activation(out=h2[:, :rows], in_=h2_ps[:, :rows],
                                         func=mybir.ActivationFunctionType.Lrelu,
                                         bias=b2_t[:], scale=1.0, alpha=0.01)
                    egoNT = wp.tile([DO, BW], F32, tag="egoNT")
                    nc.vector.tensor_tensor(out=egoNT[:, :rows], in0=h1[:, :rows],
                                            in1=h2[:, :rows], op=mybir.AluOpType.add)

                    ego_ps = pp2.tile([BW, DO], F32, space="PSUM", tag="egor")
                    nc.tensor.transpose(out=ego_ps[:rows, :], in_=egoNT[:, :rows],
                                        identity=ident[:])
                    nc.vector.tensor_copy(stage_e[:rows, b * DO : (b + 1) * DO],
                                          ego_ps[:rows, :])

            # ---- batched L2 normalize over the staged ego rows ----
            nc.vector.tensor_tensor(out=stage_n[:], in0=stage_e[:], in1=stage_e[:],
                                    op=mybir.AluOpType.mult)
            nc.vector.tensor_reduce(
                out=ss[:],
                in_=stage_n[:].rearrange("p (b d) -> p b d", d=DO),
                axis=mybir.AxisListType.X, op=mybir.AluOpType.add)
            nc.scalar.sqrt(nrm[:], ss[:])
            nc.vector.tensor_scalar_max(nrm[:], nrm[:], 1e-12)
            nc.vector.reciprocal(rinv[:], nrm[:])
            for b in range(NBLK):
                rows = BW if b < NBLK - 1 else LAST_ROWS
                nc.vector.tensor_scalar_mul(
                    stage_n[:rows, b * DO : (b + 1) * DO],
                    stage_e[:rows, b * DO : (b + 1) * DO],
                    rinv[:rows, b : b + 1])

            # ---- bulk output DMAs ----
            nc.sync.dma_start(
                norm_out[0 : NFULL * BW, :].rearrange("(b p) d -> p b d", p=BW),
                stage_n[:, : NFULL * DO].rearrange("p (b d) -> p b d", d=DO))
            nc.sync.dma_start(
                norm_out[NFULL * BW : SHARD, :],
                stage_n[:LAST_ROWS, NFULL * DO : NBLK * DO])
            if emit_ego:
                nc.sync.dma_start(
                    ego_out[0 : NFULL * BW, :].rearrange("(b p) d -> p b d", p=BW),
                    stage_e[:, : NFULL * DO].rearrange("p (b d) -> p b d", d=DO))
                nc.sync.dma_start(
                    ego_out[NFULL * BW : SHARD, :],
                    stage_e[:LAST_ROWS, NFULL * DO : NBLK * DO])

    nc.compile()
    return nc


def kernel(node_embed, edge_row, edge_col, edge_val,
           W1_0, b1_0, W2_0, b2_0, W1_1, b1_1, W2_1, b2_1):
    node_embed = np.asarray(node_embed, np.float32)
    edge_row = np.asarray(edge_row, np.int64)
    edge_col = np.asarray(edge_col, np.int64)
    edge_val = np.asarray(edge_val, np.float32)

    idx16, mval, groups, TOT = _prep_edges(edge_row, edge_col, edge_val)

    mkey = (TOT, tuple(g["tiles_g"] for g in groups))
    if ("L0", mkey) not in _cache:
        _cache[("L0", mkey)] = _build_layer(64, 32, groups, TOT, emit_ego=True)
    if ("L1", mkey) not in _cache:
        _cache[("L1", mkey)] = _build_layer(32, 16, groups, TOT, emit_ego=False)
    nc0 = _cache[("L0", mkey)]
    nc1 = _cache[("L1", mkey)]

    x0_bf = node_embed.astype(BF_NP)
    x2_0 = np.ascontiguousarray(np.concatenate([x0_bf, x0_bf], axis=1))
    in_maps0 = []
    for c in range(NC):
        in_maps0.append({
            "x2": x2_0,
            "xT": np.ascontiguousarray(x0_bf[c * SHARD : (c + 1) * SHARD].T),
            "idx16": idx16[c], "mval": mval[c],
            "w1": np.ascontiguousarray(np.asarray(W1_0, np.float32).astype(BF_NP)),
            "w2": np.ascontiguousarray(np.asarray(W2_0, np.float32).astype(BF_NP)),
            "b1": np.ascontiguousarray(np.asarray(b1_0, np.float32).reshape(-1, 1)),
            "b2": np.ascontiguousarray(np.asarray(b2_0, np.float32).reshape(-1, 1)),
        })
    res0 = run_bass_kernel_spmd(nc0, in_maps0, core_ids=list(range(NC)), trace=_TRACE)

    ego1_bf = np.ascontiguousarray(
        np.concatenate([np.asarray(res0.results[c]["ego_out"]) for c in range(NC)],
                       axis=0))
    norm1 = np.concatenate([res0.results[c]["norm_out"] for c in range(NC)], axis=0)

    x2_1 = np.ascontiguousarray(np.concatenate([ego1_bf] * 4, axis=1))
    in_maps1 = []
    for c in range(NC):
        in_maps1.append({
            "x2": x2_1,
            "xT": np.ascontiguousarray(ego1_bf[c * SHARD : (c + 1) * SHARD].T),
            "idx16": idx16[c], "mval": mval[c],
            "w1": np.ascontiguousarray(np.asarray(W1_1, np.float32).astype(BF_NP)),
            "w2": np.ascontiguousarray(np.asarray(W2_1, np.float32).astype(BF_NP)),
            "b1": np.ascontiguousarray(np.asarray(b1_1, np.float32).reshape(-1, 1)),
            "b2": np.ascontiguousarray(np.asarray(b2_1, np.float32).reshape(-1, 1)),
        })
    res1 = run_bass_kernel_spmd(nc1, in_maps1, core_ids=list(range(NC)), trace=_TRACE)
    norm2 = np.concatenate([res1.results[c]["norm_out"] for c in range(NC)], axis=0)

    global LAST_EXEC_NS
    if res0.exec_time_ns is not None or res1.exec_time_ns is not None:
        LAST_EXEC_NS = (res0.exec_time_ns or 0) + (res1.exec_time_ns or 0)
        globals()["LAST_RES"] = (res0, res1)

    out = np.empty((N, 64 + 32 + 16), np.float32)
    out[:, :64] = node_embed
    out[:, 64:96] = norm1
    out[:, 96:] = norm2
    return out
